# revision 1
# baseline (speedup 1.0000x reference)
"""Trainium2 Bass kernel for nn_LocationSlayerRandom (SLAYER two-branch spiking net).

Contract: kernel(**inputs) takes the FULL unsharded inputs
  spike_input [32,156,1,1,2048] f32, W1 [512,156], W2 [20,512],
  Wl1 [512,2048], Wl2 [20,512], perm [156] i32
and returns the FULL output [32,20,1,1,2204] f32.

Strategy (8 cores, data-parallel over batch, 4 samples/core):

Branch 1 (per sample b):
  u1 = psp_t(W1 @ si) = W1 @ psp_t(si)            (psp is linear => commutes)
  - psp_t(si): DVE tensor_tensor_scan along t on the 156-row input
    (channels 0:127 per-b slices of one packed tile; channels 128:155 of all
    4 b packed into one 128-partition tile at offsets 32b, with per-sample
    zero-masked 128-row weight tiles selecting each sample's rows).
  - fc1 on PE (bf16), threshold fused straight from PSUM into fp8 encodings
    of s1 - 0.5: hidden m-tiles 0..2 on ACT as Sign(u1-10) in {-1,0,1} with
    fc2 weights pre-scaled 0.5; m-tile 3 on DVE as (u1>=10)-0.5 in {-.5,.5}
    with unscaled weights (engine load balancing). The affine
    0.5*rowsum(W2) correction is folded into a host-side time-varying
    threshold T2[o,t] = 10 - 0.5*rowsum(W2_eff)[o]*g[t],
    g[t] = sum_{k<=t} alpha^k.
  - fc2 on PE in fp8, with the four samples packed into the four PE column
    groups (tile_position=(0,32b)) accumulating into ONE [128,2048] PSUM
    tile; one psp scan straight from PSUM; spike_output = (v >= T2).

Branch 2: ul1 = psp_c'(Wl1 @ x_tp) where x_tp[b,t,c'] = si[b,perm[c'],t].
  Host supplies the gathered+transposed input tiles sipT (pure layout prep),
  so the t-contraction runs with Wl1^T stationary and the c'-psp becomes a
  free-dim scan straight from PSUM with a reset-pattern data0 (alpha, but 0
  at each sample boundary). Then threshold, fc2, scan, threshold.

Numerics: matmuls bf16 (fc2-b1 fp8) with fp32 accumulate. The only
nonlinearity is the >=10 threshold; true layer-2 potentials sit below 3.2
(branch 1) / 2.0 (branch 2) against a threshold of 10, so near-threshold
layer-1 bit flips from low-precision weights cannot flip any output bit.
"""

from contextlib import ExitStack

import numpy as np
import ml_dtypes

import concourse.bass as bass
import concourse.mybir as mybir
from concourse import bacc
from concourse import tile as tile_mod
from concourse.bass_utils import run_bass_kernel_spmd

F32 = mybir.dt.float32
BF16 = mybir.dt.bfloat16
FP8 = mybir.dt.float8e4
AL = mybir.AluOpType
AF = mybir.ActivationFunctionType
BF16_NP = ml_dtypes.bfloat16
FP8_NP = ml_dtypes.float8_e4m3

B, C_IN, T = 32, 156, 2048
HID, OUT_DIM = 512, 20
CP = 156                      # permuted taxel axis (branch-2 "time")
N_CORES = 8
B_PER = B // N_CORES          # 4 samples per core
ALPHA = float(np.exp(-1.0 / 10.0))
THETA = 10.0
NB2 = B_PER * CP              # 624, branch-2 packed free dim
KT = T // 128                 # 16 k-tiles over t


def build_program(tc, outs, ins):
    nc = tc.nc
    out = outs["out"]

    with ExitStack() as ctx:
        consts = ctx.enter_context(tc.tile_pool(name="consts", bufs=1))
        work = ctx.enter_context(tc.tile_pool(name="work", bufs=1))
        sgp = ctx.enter_context(tc.tile_pool(name="sgp", bufs=16))
        mid = ctx.enter_context(tc.tile_pool(name="mid", bufs=4))
        psum1 = ctx.enter_context(tc.tile_pool(name="psum1", bufs=4, space="PSUM"))

        # ---------------- constant patterns (gpsimd; SBUF only) ----------
        alpha_t = consts.tile([128, T], F32, tag="alpha")
        nc.gpsimd.memset(alpha_t[:], ALPHA)
        pat624 = consts.tile([128, NB2], F32, tag="pat624")
        nc.gpsimd.memset(pat624[:], ALPHA)
        for j in range(B_PER):
            nc.gpsimd.memset(pat624[:, j * CP:j * CP + 1], 0.0)
        bias_m10 = consts.tile([128, 1], F32, tag="bm10")
        nc.gpsimd.memset(bias_m10[:], -THETA)
        act_warm = consts.tile([128, 1], F32, tag="actwarm")
        nc.scalar.activation(act_warm[:], bias_m10[:], AF.Sign,
                             bias=bias_m10[:])

        # ---------------- inputs (consolidated DMAs) ---------------------
        # branch-1 critical path first
        siA = consts.tile([128, B_PER * T], BF16, tag="siA")
        siB = consts.tile([128, T], BF16, tag="siB")
        nc.sync.dma_start(siA[:, 0:T], ins["siA"][:, 0:T])
        nc.sync.dma_start(siB[:], ins["siB"][:])
        # branch-2 fp8 inputs early: A1(m0) runs during the scan ramp
        wl1 = consts.tile([128, KT * HID], FP8, tag="wl1")
        nc.sync.dma_start(wl1[:], ins["Wl1T"][:])
        sip = consts.tile([128, KT * NB2], FP8, tag="sip")
        nc.sync.dma_start(sip[:], ins["sipT"][:])
        for b in range(1, B_PER):        # per-sample slices so scan b starts
            nc.sync.dma_start(siA[:, b * T:(b + 1) * T],   # after its own DMA
                              ins["siA"][:, b * T:(b + 1) * T])
        w1a = consts.tile([128, HID], BF16, tag="w1a")
        nc.sync.dma_start(w1a[:], ins["W1Ta"][:])
        w1b = consts.tile([128, B_PER * HID], BF16, tag="w1b")
        nc.sync.dma_start(w1b[:], ins["W1Tb"][:])
        w2p = consts.tile([128, 4 * 32], FP8, tag="w2p")
        nc.sync.dma_start(w2p[:], ins["W2pT"][:])
        t2_t = consts.tile([128, T], F32, tag="t2")
        nc.sync.dma_start(t2_t[:], ins["T2"][:])
        wl2 = consts.tile([128, 4 * OUT_DIM], BF16, tag="wl2")
        nc.sync.dma_start(wl2[:], ins["Wl2T"][:])

        # ---------------- branch-1 input psp scans (DVE) -----------------
        # order: the packed tail tile first, then sample 0 (fc1 b0 needs both
        # before its first accumulation group completes), then samples 1..3.
        # Emitting the early-needed scans first also keeps their completion
        # events early in the DVE stream (a later event would stall fc1).
        psA = work.tile([128, B_PER * T], BF16, tag="psA")
        psB = work.tile([128, T], BF16, tag="psB")
        nc.vector.tensor_tensor_scan(psA[:, 0:T], alpha_t[:],
                                     siA[:, 0:T], 0.0, AL.mult, AL.add)
        nc.vector.tensor_tensor_scan(psB[:], alpha_t[:], siB[:], 0.0,
                                     AL.mult, AL.add)

        # ---------------- branch 2 A1 block emitter (interleaved below) --
        # fp8 DoubleRow: two 128-row k-subtiles per pass ([128, 2, X] APs),
        # halving the pass count of the t-contraction. Emitted one m-block
        # after each fc1 sample so the PE fills fc1's threshold-paced gaps.
        wl1_3d = wl1[:].rearrange("p (k o) -> p k o", o=HID)
        sip_3d = sip[:].rearrange("p (k c) -> p k c", c=NB2)
        DR = mybir.MatmulPerfMode.DoubleRow
        l1 = []

        def a1_block(m):
            pa = psum1.tile([128, 1024], F32, tag="psum1", name=f"pa{m}")
            a1 = pa[:, :NB2]
            msl = slice(m * 128, (m + 1) * 128)
            for ki in range(KT // 2):
                st, sp = (ki == 0), (ki == KT // 2 - 1)
                lhs = wl1_3d[:, 2 * ki:2 * ki + 2, msl]
                nc.tensor.matmul(a1[:, 0:512], lhs,
                                 sip_3d[:, 2 * ki:2 * ki + 2, 0:512],
                                 start=st, stop=sp, perf_mode=DR)
                nc.tensor.matmul(a1[:, 512:NB2], lhs,
                                 sip_3d[:, 2 * ki:2 * ki + 2, 512:NB2],
                                 start=st, stop=sp, perf_mode=DR)
            u = mid.tile([128, NB2], F32, tag="ul1", name=f"ul1{m}")
            nc.vector.tensor_tensor_scan(u[:], pat624[:], a1, 0.0,
                                         AL.mult, AL.add)
            lt = mid.tile([128, NB2], BF16, tag="l1", name=f"l1{m}")
            nc.vector.tensor_scalar(lt[:], u[:], THETA, None, AL.is_ge)
            l1.append(lt)

        # A1(m0) fills the PE-idle scan ramp; its scan slots into the DVE
        # stream before the remaining si scans.
        a1_block(0)
        for b in range(1, B_PER):
            nc.vector.tensor_tensor_scan(psA[:, b * T:(b + 1) * T], alpha_t[:],
                                         siA[:, b * T:(b + 1) * T], 0.0,
                                         AL.mult, AL.add)

        # ---------------- branch 1 fc1 + fused Sign thresholds -----------
        # loop order b -> half -> m: the whole first inner phase consumes only
        # the first-half scans, so fc1 never stalls on a later half-scan.
        sgt = {}
        for b in range(B_PER):
            if b >= 2:
                a1_block(b - 1)
            for m in range(4):
                sgt[(b, m)] = sgp.tile([128, T], FP8, tag="sg", name=f"sg{b}{m}")
            for m in range(4):
                for half in range(2):
                    s_t = sgt[(b, m)]
                    msl = slice(m * 128, (m + 1) * 128)
                    bmsl = slice(b * HID + m * 128, b * HID + (m + 1) * 128)
                    pu = psum1.tile([128, 1024], F32, tag="psum1")
                    for ch in range(2):
                        tsl = slice(b * T + half * 1024 + ch * 512,
                                    b * T + half * 1024 + (ch + 1) * 512)
                        nc.tensor.matmul(pu[:, ch * 512:(ch + 1) * 512],
                                         w1a[:, msl], psA[:, tsl],
                                         start=True, stop=False)
                    for ch in range(2):
                        tsl = slice(half * 1024 + ch * 512,
                                    half * 1024 + (ch + 1) * 512)
                        nc.tensor.matmul(pu[:, ch * 512:(ch + 1) * 512],
                                         w1b[:, bmsl], psB[:, tsl],
                                         start=False, stop=True)
                    hsl = slice(half * 1024, (half + 1) * 1024)
                    if m < 3:
                        # ACT: sg = sign(u1-10) in {-1,0,1}; weights 0.5x
                        nc.scalar.activation(s_t[:, hsl], pu[:], AF.Sign,
                                             bias=bias_m10[:])
                    else:
                        # DVE: sg = (u1>=10)-0.5 in {-.5,.5}; weights 1.0x
                        nc.vector.tensor_scalar(s_t[:, hsl], pu[:], THETA, 0.5,
                                                AL.is_ge, AL.subtract)

        a1_block(3)

        # branch 2 fc2 + psp + threshold + out (emitted BEFORE branch-1 fc2:
        # its l1 inputs are ready during fc1 thanks to the A1 interleave, so
        # its whole chain hides under the fc2-b1 matmuls below)
        pl2full = psum1.tile([128, 1024], F32, tag="psum1")
        pl2 = pl2full[:OUT_DIM, :NB2]
        for k in range(4):
            st, sp = (k == 0), (k == 3)
            ksl = slice(k * OUT_DIM, (k + 1) * OUT_DIM)
            nc.tensor.matmul(pl2[:, 0:512], wl2[:, ksl], l1[k][:, 0:512],
                             start=st, stop=sp)
            nc.tensor.matmul(pl2[:, 512:NB2], wl2[:, ksl], l1[k][:, 512:NB2],
                             start=st, stop=sp)
        ul2 = mid.tile([128, NB2], F32, tag="ul2")
        nc.vector.tensor_tensor_scan(ul2[:OUT_DIM], pat624[:OUT_DIM], pl2, 0.0,
                                     AL.mult, AL.add)
        o2 = mid.tile([128, NB2], F32, tag="o2")
        nc.vector.tensor_scalar(o2[:OUT_DIM], ul2[:OUT_DIM], THETA, None,
                                AL.is_ge)
        nc.sync.dma_start(
            out[:, :OUT_DIM, T:T + CP].rearrange("b o c -> o b c"),
            o2[:OUT_DIM, :].rearrange("o (b c) -> o b c", c=CP))

        # ---------------- branch 1 fc2, col-tiled over samples -----------
        # t-half at a time: the first half's scan + compare + output DMA
        # overlap the second half's matmuls, shrinking the kernel tail.
        vs = work.tile([128, T], F32, tag="vs")
        o1 = work.tile([128, T], F32, tag="o1")
        prev_init = 0.0
        for hf in range(2):
            puh = psum1.tile([128, 1024], F32, tag="psum1", name=f"pu2{hf}")
            for k in range(4):
                ksl = slice(k * 32, k * 32 + 32)
                for b in range(B_PER):
                    for ch in range(2):
                        csl = slice(hf * 1024 + ch * 512,
                                    hf * 1024 + (ch + 1) * 512)
                        nc.tensor.matmul(puh[32 * b:32 * b + 32,
                                             ch * 512:(ch + 1) * 512],
                                         w2p[:, ksl], sgt[(b, k)][:, csl],
                                         start=(k == 0), stop=(k == 3),
                                         tile_position=(0, 32 * b),
                                         skip_group_check=True)
            hsl = slice(hf * 1024, (hf + 1) * 1024)
            nc.vector.tensor_tensor_scan(vs[:, hsl], alpha_t[:, 0:1024],
                                         puh[:], prev_init, AL.mult, AL.add)
            prev_init = vs[:, hf * 1024 + 1023:hf * 1024 + 1024]
            nc.vector.tensor_tensor(o1[:, hsl], vs[:, hsl], t2_t[:, hsl],
                                    AL.is_ge)
            nc.sync.dma_start(
                out[:, :, hsl].rearrange("b j t -> (b j) t"), o1[:, hsl])


# ======================= host-side preparation =======================

def prep_core_inputs(si, sip, core):
    """Per-core data tensors, pre-packed into single-DMA SBUF layouts.
    si/sip are [32,156,2048] f32 (sip already perm-gathered)."""
    sl = si[core * B_PER:(core + 1) * B_PER]          # [4,156,2048]
    # siA [128, 4*T]: [p, b*T+t] = si[b, p, t]
    siA = np.ascontiguousarray(
        sl[:, :128, :].transpose(1, 0, 2).reshape(128, B_PER * T)
    ).astype(BF16_NP)
    siB = np.zeros((128, T), dtype=BF16_NP)
    for b in range(B_PER):
        siB[32 * b:32 * b + (C_IN - 128)] = sl[b, 128:C_IN, :]
    sp = sip[core * B_PER:(core + 1) * B_PER]         # [4,156,2048]
    # sipT [128, KT*NB2]: [p, k*NB2 + b*CP + c'] = sip[b, c', 128k+p]
    sipT = np.ascontiguousarray(
        sp.transpose(2, 0, 1).reshape(KT, 128, NB2)
        .transpose(1, 0, 2).reshape(128, KT * NB2)
    ).astype(FP8_NP)
    return {"siA": siA, "siB": siB, "sipT": sipT}


def prep_shared_inputs(W1, W2, Wl1, Wl2):
    """Weight layouts + threshold tensor, shared by all cores."""
    w1t = np.zeros((160, HID), dtype=np.float32)
    w1t[:C_IN] = W1.T
    W1Ta = w1t[:128].astype(BF16_NP)
    # Tail channels 128:155 live at partitions 32b..32b+27 of the packed psB
    # tile; per-sample 128-row weight tiles, zero outside the sample's rows,
    # packed [128, 4*HID] with [p, b*HID+o].
    W1Tb = np.zeros((128, B_PER * HID), dtype=BF16_NP)
    for b in range(B_PER):
        W1Tb[32 * b:32 * b + 32, b * HID:(b + 1) * HID] = \
            w1t[128:160].astype(BF16_NP)

    # fc2 weights, fp8, padded to 32 cols per k-tile so the col-tiled
    # matmuls initialize full 32-row PSUM stripes. Per-k scale matches the
    # sg encoding of hidden m-tile k: ACT Sign (+-1) -> 0.5x, DVE (+-.5)
    # -> 1.0x. Layout [128, 4*32]: [p, k*32+o]
    k_scale = (0.5, 0.5, 0.5, 1.0)
    w2t = W2.T.astype(np.float32)                     # [512, 20]
    W2pT = np.zeros((128, 4 * 32), dtype=FP8_NP)
    for k in range(4):
        W2pT[:, k * 32:k * 32 + OUT_DIM] = (
            k_scale[k] * w2t[k * 128:(k + 1) * 128]).astype(FP8_NP)
    # effective (device) W2 after fp8 rounding, unscaled
    w2_eff = np.empty((HID, OUT_DIM), dtype=np.float32)
    for k in range(4):
        w2_eff[k * 128:(k + 1) * 128] = (
            W2pT[:, k * 32:k * 32 + OUT_DIM].astype(np.float32) / k_scale[k]
        )
    r2 = w2_eff.sum(axis=0)                           # [20]
    g = (1.0 - ALPHA ** (np.arange(T, dtype=np.float64) + 1)) / (1.0 - ALPHA)
    theta2 = (THETA - 0.5 * np.outer(r2, g)).astype(np.float32)   # [20, T]
    T2 = np.full((128, T), 1e30, dtype=np.float32)
    for b in range(B_PER):
        T2[32 * b:32 * b + OUT_DIM] = theta2

    # Wl1T [128, KT*HID]: [p, k*HID+o] = Wl1[o, 128k+p]
    Wl1T = np.ascontiguousarray(
        Wl1.T.reshape(KT, 128, HID).transpose(1, 0, 2).reshape(128, KT * HID)
    ).astype(FP8_NP)
    # Wl2T [128, 4*OUT]: [p, k*OUT+o] = Wl2[o, 128k+p]
    Wl2T = np.ascontiguousarray(
        Wl2.T.reshape(4, 128, OUT_DIM).transpose(1, 0, 2).reshape(128, 4 * OUT_DIM)
    ).astype(BF16_NP)
    return {"W1Ta": W1Ta, "W1Tb": W1Tb, "W2pT": W2pT, "Wl1T": Wl1T,
            "Wl2T": Wl2T, "T2": T2}


def make_in_maps(spike_input, W1, W2, Wl1, Wl2, perm):
    si = np.asarray(spike_input, dtype=np.float32).reshape(B, C_IN, T)
    perm = np.asarray(perm).astype(np.int64)
    sip = si[:, perm, :]                              # perm-gather (layout only)
    shared = prep_shared_inputs(np.asarray(W1, np.float32),
                                np.asarray(W2, np.float32),
                                np.asarray(Wl1, np.float32),
                                np.asarray(Wl2, np.float32))
    in_maps = []
    for core in range(N_CORES):
        m = dict(shared)
        m.update(prep_core_inputs(si, sip, core))
        in_maps.append(m)
    return in_maps


_IN_SPECS = {
    "siA": ((128, B_PER * T), BF16),
    "siB": ((128, T), BF16),
    "sipT": ((128, KT * NB2), FP8),
    "W1Ta": ((128, HID), BF16),
    "W1Tb": ((128, B_PER * HID), BF16),
    "W2pT": ((128, 4 * 32), FP8),
    "Wl1T": ((128, KT * HID), FP8),
    "Wl2T": ((128, 4 * OUT_DIM), BF16),
    "T2": ((128, T), F32),
}


def build_bass():
    nc = bacc.Bacc("TRN2", target_bir_lowering=False, debug=False)
    ins = {}
    for name, (shape, dt) in _IN_SPECS.items():
        h = nc.dram_tensor(name, list(shape), dt, kind="ExternalInput")
        ins[name] = h[:]
    out_h = nc.dram_tensor("out", [B_PER, 32, T + CP], F32,
                           kind="ExternalOutput")
    outs = {"out": out_h[:]}
    with tile_mod.TileContext(nc) as tc:
        build_program(tc, outs, ins)
    nc.compile()
    return nc


_NC_CACHE = None


def run(inputs, trace=False, **kw):
    """Run on the 8 NeuronCores; returns (full_output, BassKernelResults)."""
    global _NC_CACHE
    if _NC_CACHE is None:
        _NC_CACHE = build_bass()
    nc = _NC_CACHE
    in_maps = make_in_maps(**inputs)
    res = run_bass_kernel_spmd(nc, in_maps, core_ids=list(range(N_CORES)),
                               trace=trace, **kw)
    parts = [res.results[c]["out"][:, :OUT_DIM, :] for c in range(N_CORES)]
    full = np.concatenate(parts, axis=0).reshape(B, OUT_DIM, 1, 1, T + CP)
    return np.ascontiguousarray(full.astype(np.float32)), res


def kernel(**inputs):
    out, _ = run(inputs)
    return out



# revision 6
# speedup vs baseline: 1.1203x; 1.1203x over previous
"""Trainium2 Bass kernel for nn_LocationSlayerRandom (SLAYER two-branch spiking net).

Contract: kernel(**inputs) takes the FULL unsharded inputs
  spike_input [32,156,1,1,2048] f32, W1 [512,156], W2 [20,512],
  Wl1 [512,2048], Wl2 [20,512], perm [156] i32
and returns the FULL output [32,20,1,1,2204] f32.

Strategy (8 cores, data-parallel over batch, 4 samples/core), v2:

Branch 1 (per sample b):  u1 = W1 @ psp_t(si)  (psp is linear => commutes)
  - psp_t(si): DVE tensor_tensor_scan along t (fp8 spike input, bf16 out),
    split into t-halves for pipelining; scans chain via initial=prev last col.
  - bf16 -> fp8 casts (DVE tensor_copy, plus two early ones on ACT) feed
  - fc1 on PE in fp8 DoubleRow: the (128-ch, 28-ch-tail) contraction pair in
    ONE 256-row pass via a step-sliced [128,2,512] AP over a packed ps8 tile.
  - thresholds all on ACT as Sign(u1-10) in {-1,0,1} fp8; fc2 weights
    pre-scaled 0.5 and the affine 0.5*rowsum(W2) folded into a host-side
    time-varying threshold T2[o,t] (bf16).
  - fc2 on PE fp8 with 4 samples in the 4 PE column groups -> one
    [128,1024] PSUM per t-half; psp scan from PSUM (bf16 out); compare vs
    T2 (all-bf16 2x DVE); bf16 output DMA.

Branch 2: ul1 = psp_c'(Wl1 @ x_tp), x_tp host-gathered/transposed (sipT).
  - A1 m-blocks on PE (fp8 DoubleRow over t), interleaved with fc1 to fill
    threshold-paced PE gaps; c'-psp scan straight from PSUM with the
    reset-pattern multiplier; l1 threshold on ACT Sign (wl2 0.5-scaled,
    rowsum correction in T2b).
  - locationFc2 col-tiled over samples: psum [32b+o, c'], so its psp scan is
    a [128,156] scan (4x shorter than [20,624]) and needs no reset pattern.

Numerics: all heavy matmuls fp8 with fp32 accumulate; psp states bf16.
The only nonlinearity is the >=10 threshold; true layer-2 potentials sit
below 3.2 (branch 1) / 2.0 (branch 2) against a threshold of 10, so
near-threshold layer-1 bit flips from low-precision weights/activations
cannot flip any output bit.
"""

from contextlib import ExitStack

import numpy as np
import ml_dtypes

import concourse.bass as bass
import concourse.mybir as mybir
from concourse import bacc
from concourse import tile as tile_mod
from concourse.bass_utils import run_bass_kernel_spmd

F32 = mybir.dt.float32
BF16 = mybir.dt.bfloat16
FP8 = mybir.dt.float8e4
AL = mybir.AluOpType
AF = mybir.ActivationFunctionType
BF16_NP = ml_dtypes.bfloat16
FP8_NP = ml_dtypes.float8_e4m3

B, C_IN, T = 32, 156, 2048
HID, OUT_DIM = 512, 20
CP = 156                      # permuted taxel axis (branch-2 "time")
N_CORES = 8
B_PER = B // N_CORES          # 4 samples per core
ALPHA = float(np.exp(-1.0 / 10.0))
THETA = 10.0
NB2 = B_PER * CP              # 624, branch-2 packed free dim
KT = T // 128                 # 16 k-tiles over t
H = T // 2                    # 1024, t-half


def build_program(tc, outs, ins):
    nc = tc.nc
    out = outs["out"]
    DR = mybir.MatmulPerfMode.DoubleRow

    with ExitStack() as ctx:
        consts = ctx.enter_context(tc.tile_pool(name="consts", bufs=1))
        work = ctx.enter_context(tc.tile_pool(name="work", bufs=1))
        psum = ctx.enter_context(tc.tile_pool(name="psum", bufs=4, space="PSUM"))

        # ---------------- constant patterns (gpsimd; SBUF only) ----------
        alpha_t = consts.tile([128, T], F32, tag="alpha")
        nc.gpsimd.memset(alpha_t[:], ALPHA)
        pat624 = consts.tile([128, NB2], F32, tag="pat624")
        nc.gpsimd.memset(pat624[:], ALPHA)
        for j in range(B_PER):
            nc.gpsimd.memset(pat624[:, j * CP:j * CP + 1], 0.0)
        bias_m10 = consts.tile([128, 1], F32, tag="bm10")
        nc.gpsimd.memset(bias_m10[:], -THETA)
        act_warm = consts.tile([128, 1], F32, tag="actwarm")
        nc.scalar.activation(act_warm[:], bias_m10[:], AF.Sign,
                             bias=bias_m10[:])

        # ---------------- inputs (ordered for earliest need) -------------
        siB = consts.tile([128, T], FP8, tag="siB")
        nc.sync.dma_start(siB[:], ins["siB"][:])
        siA = consts.tile([128, B_PER * T], FP8, tag="siA")
        nc.sync.dma_start(siA[:, 0:H], ins["siA"][:, 0:H])
        nc.sync.dma_start(siA[:, H:T], ins["siA"][:, H:T])
        w1dr = consts.tile([128, B_PER * 2 * HID], FP8, tag="w1dr")
        nc.sync.dma_start(w1dr[:], ins["W1dr"][:])
        sip = consts.tile([128, KT * NB2], FP8, tag="sip")
        nc.sync.dma_start(sip[:], ins["sipT"][:])
        wl1 = consts.tile([128, KT * HID], FP8, tag="wl1")
        nc.sync.dma_start(wl1[:], ins["Wl1T"][:])
        for b in range(1, B_PER):
            nc.sync.dma_start(siA[:, b * T:(b + 1) * T],
                              ins["siA"][:, b * T:(b + 1) * T])
        t2_t = consts.tile([128, T], BF16, tag="t2")
        nc.sync.dma_start(t2_t[:], ins["T2"][:])
        w2p = consts.tile([128, 4 * 32], FP8, tag="w2p")
        nc.sync.dma_start(w2p[:], ins["W2pT"][:])
        wl2 = consts.tile([128, 4 * 32], BF16, tag="wl2")
        nc.sync.dma_start(wl2[:], ins["Wl2T"][:])
        t2b = consts.tile([128, CP], BF16, tag="t2b")
        nc.sync.dma_start(t2b[:], ins["T2b"][:])

        # ---------------- persistent work tiles --------------------------
        psA = work.tile([128, B_PER * T], BF16, tag="psA")
        psB = work.tile([128, T], BF16, tag="psB")
        ps8 = work.tile([128, 5 * T], FP8, tag="ps8")
        ps8_3d = ps8[:].rearrange("p (n t) -> p n t", n=5)
        w1_4d = w1dr[:].rearrange("p (b k o) -> p b k o", b=B_PER, k=2)
        wl1_3d = wl1[:].rearrange("p (k o) -> p k o", o=HID)
        sip_3d = sip[:].rearrange("p (k c) -> p k c", c=NB2)
        sg = [work.tile([128, 4 * T], FP8, tag=f"sg{b}", name=f"sg{b}")
              for b in range(B_PER)]
        sg3 = [s[:].rearrange("p (m t) -> p m t", m=4) for s in sg]
        ul1 = [work.tile([128, NB2], F32, tag=f"ul1{m}", name=f"ul1{m}")
               for m in range(4)]
        l1 = [work.tile([128, NB2], BF16, tag=f"l1{m}", name=f"l1{m}")
              for m in range(4)]
        vs = work.tile([128, T], BF16, tag="vs")
        o1 = work.tile([128, T], BF16, tag="o1")
        vs2 = work.tile([128, CP], BF16, tag="vs2")
        o2 = work.tile([128, CP], BF16, tag="o2")

        # ---------------- emission helpers -------------------------------
        def scanA(b, h):
            sl = slice(b * T + h * H, b * T + (h + 1) * H)
            init = 0.0 if h == 0 else psA[:, b * T + H - 1:b * T + H]
            nc.vector.tensor_tensor_scan(psA[:, sl], alpha_t[:, 0:H],
                                         siA[:, sl], init, AL.mult, AL.add)

        def scanB(h):
            sl = slice(h * H, (h + 1) * H)
            init = 0.0 if h == 0 else psB[:, H - 1:H]
            nc.vector.tensor_tensor_scan(psB[:, sl], alpha_t[:, 0:H],
                                         siB[:, sl], init, AL.mult, AL.add)

        def cast(block, src, h, engine):
            sl = slice(h * H, (h + 1) * H)
            dst = ps8[:, block * T + h * H:block * T + (h + 1) * H]
            if engine == "act":
                nc.scalar.activation(dst, src[:, sl], AF.Copy)
            else:
                nc.vector.tensor_copy(dst, src[:, sl])

        def castA(b, h, engine):
            sl = slice(b * T + h * H, b * T + (h + 1) * H)
            dst = ps8[:, b * T + h * H:b * T + (h + 1) * H]
            if engine == "act":
                nc.scalar.activation(dst, psA[:, sl], AF.Copy)
            else:
                nc.vector.tensor_copy(dst, psA[:, sl])

        fc1_psum = {}

        def fc1(b, h):
            # fp8 DoubleRow: (psA_b block, psB block) pair via step-sliced AP
            for m in range(4):
                pu = psum.tile([128, 1024], F32, tag="psum",
                               name=f"pu{b}{h}{m}")
                lhs = w1_4d[:, b, :, m * 128:(m + 1) * 128]
                for ch in range(2):
                    csl = slice(h * H + ch * 512, h * H + (ch + 1) * 512)
                    rhs = ps8_3d[:, b:5:(4 - b), csl]
                    nc.tensor.matmul(pu[:, ch * 512:(ch + 1) * 512], lhs, rhs,
                                     start=True, stop=True, perf_mode=DR)
                fc1_psum[(b, h, m)] = pu

        def thr(b, h):
            hs = slice(h * H, (h + 1) * H)
            for m in range(4):
                nc.scalar.activation(sg3[b][:, m, hs], fc1_psum[(b, h, m)][:],
                                     AF.Sign, bias=bias_m10[:])

        a1_psum = {}

        def a1_mm(m):
            pa = psum.tile([128, 1024], F32, tag="psum", name=f"pa{m}")
            a1 = pa[:, :NB2]
            msl = slice(m * 128, (m + 1) * 128)
            for ki in range(KT // 2):
                st, sp = (ki == 0), (ki == KT // 2 - 1)
                lhs = wl1_3d[:, 2 * ki:2 * ki + 2, msl]
                nc.tensor.matmul(a1[:, 0:512], lhs,
                                 sip_3d[:, 2 * ki:2 * ki + 2, 0:512],
                                 start=st, stop=sp, perf_mode=DR)
                nc.tensor.matmul(a1[:, 512:NB2], lhs,
                                 sip_3d[:, 2 * ki:2 * ki + 2, 512:NB2],
                                 start=st, stop=sp, perf_mode=DR)
            a1_psum[m] = a1

        def a1_scan(m):
            nc.vector.tensor_tensor_scan(ul1[m][:], pat624[:], a1_psum[m],
                                         0.0, AL.mult, AL.add)

        def a1_thr(m):
            nc.scalar.activation(l1[m][:], ul1[m][:], AF.Sign,
                                 bias=bias_m10[:])

        def fc2b1(h):
            pu2 = psum.tile([128, 1024], F32, tag="psum", name=f"pu2{h}")
            for k in range(4):
                ksl = slice(k * 32, k * 32 + 32)
                for b in range(B_PER):
                    for ch in range(2):
                        csl = slice(h * H + ch * 512, h * H + (ch + 1) * 512)
                        nc.tensor.matmul(pu2[32 * b:32 * b + 32,
                                             ch * 512:(ch + 1) * 512],
                                         w2p[:, ksl], sg3[b][:, k, csl],
                                         start=(k == 0), stop=(k == 3),
                                         tile_position=(0, 32 * b),
                                         skip_group_check=True)
            return pu2

        def fc2b1_post(h, pu2):
            hs = slice(h * H, (h + 1) * H)
            init = 0.0 if h == 0 else vs[:, H - 1:H]
            nc.vector.tensor_tensor_scan(vs[:, hs], alpha_t[:, 0:H],
                                         pu2[:], init, AL.mult, AL.add)
            nc.vector.tensor_tensor(o1[:, hs], vs[:, hs], t2_t[:, hs],
                                    AL.is_ge)
            nc.sync.dma_start(
                out[:, :, hs].rearrange("b j t -> (b j) t"), o1[:, hs])

        # ================= schedule =================
        # PE ramp: A1(m0) first (DMA-paced), then fc1 interleaved with A1.
        a1_mm(0)

        # ---- phase h0 ----
        scanB(0)
        scanA(0, 0)
        cast(4, psB, 0, "act")
        castA(0, 0, "act")
        fc1(0, 0)
        thr(0, 0)
        scanA(1, 0)
        castA(1, 0, "dve")
        fc1(1, 0)
        thr(1, 0)
        scanA(2, 0)
        castA(2, 0, "dve")
        a1_scan(0)
        a1_thr(0)
        a1_mm(1)
        fc1(2, 0)
        thr(2, 0)
        scanA(3, 0)
        castA(3, 0, "dve")
        fc1(3, 0)
        thr(3, 0)
        a1_mm(2)
        pu2_h0 = fc2b1(0)

        # ---- phase h1 (DVE keeps scanning while ACT drains h0 thr) ----
        scanB(1)
        scanA(0, 1)
        cast(4, psB, 1, "act")
        castA(0, 1, "act")
        a1_scan(1)
        a1_thr(1)
        fc2b1_post(0, pu2_h0)
        fc1(0, 1)
        thr(0, 1)
        a1_mm(3)
        scanA(1, 1)
        castA(1, 1, "dve")
        a1_scan(2)
        a1_thr(2)
        fc1(1, 1)
        thr(1, 1)
        scanA(2, 1)
        castA(2, 1, "dve")
        fc1(2, 1)
        thr(2, 1)
        scanA(3, 1)
        castA(3, 1, "dve")
        a1_scan(3)
        a1_thr(3)
        fc1(3, 1)
        thr(3, 1)

        # branch-2 fc2, col-tiled over samples: psum rows 32b+o, free c'
        pl2 = psum.tile([128, 1024], F32, tag="psum", name="pl2")
        for k in range(4):
            ksl = slice(k * 32, k * 32 + 32)
            for b in range(B_PER):
                nc.tensor.matmul(pl2[32 * b:32 * b + 32, 0:CP],
                                 wl2[:, ksl], l1[k][:, b * CP:(b + 1) * CP],
                                 start=(k == 0), stop=(k == 3),
                                 tile_position=(0, 32 * b),
                                 skip_group_check=True)
        nc.vector.tensor_tensor_scan(vs2[:], alpha_t[:, 0:CP], pl2[:, 0:CP],
                                     0.0, AL.mult, AL.add)
        nc.vector.tensor_tensor(o2[:], vs2[:], t2b[:], AL.is_ge)
        for b in range(B_PER):
            nc.sync.dma_start(out[b, 0:OUT_DIM, T:T + CP],
                              o2[32 * b:32 * b + OUT_DIM, :])

        # branch-1 fc2 second half (kernel tail)
        pu2_h1 = fc2b1(1)
        fc2b1_post(1, pu2_h1)


# ======================= host-side preparation =======================

def prep_core_inputs(si, sip, core):
    """Per-core data tensors, pre-packed into single-DMA SBUF layouts.
    si/sip are [32,156,2048] f32 (sip already perm-gathered)."""
    sl = si[core * B_PER:(core + 1) * B_PER]          # [4,156,2048]
    # siA [128, 4*T]: [p, b*T+t] = si[b, p, t]
    siA = np.ascontiguousarray(
        sl[:, :128, :].transpose(1, 0, 2).reshape(128, B_PER * T)
    ).astype(FP8_NP)
    siB = np.zeros((128, T), dtype=FP8_NP)
    for b in range(B_PER):
        siB[32 * b:32 * b + (C_IN - 128)] = sl[b, 128:C_IN, :]
    sp = sip[core * B_PER:(core + 1) * B_PER]         # [4,156,2048]
    # sipT [128, KT*NB2]: [p, k*NB2 + b*CP + c'] = sip[b, c', 128k+p]
    sipT = np.ascontiguousarray(
        sp.transpose(2, 0, 1).reshape(KT, 128, NB2)
        .transpose(1, 0, 2).reshape(128, KT * NB2)
    ).astype(FP8_NP)
    return {"siA": siA, "siB": siB, "sipT": sipT}


def prep_shared_inputs(W1, W2, Wl1, Wl2):
    """Weight layouts + threshold tensors, shared by all cores."""
    w1t = np.zeros((160, HID), dtype=np.float32)
    w1t[:C_IN] = W1.T
    # W1dr [128, 4*2*512]: [p, b, 0, o] = W1.T[p, o] (c 0..127);
    # [p, b, 1, o] = tail channels masked to sample b's psB rows.
    W1dr = np.zeros((128, B_PER, 2, HID), dtype=FP8_NP)
    for b in range(B_PER):
        W1dr[:, b, 0, :] = w1t[:128].astype(FP8_NP)
        W1dr[32 * b:32 * b + 32, b, 1, :] = w1t[128:160].astype(FP8_NP)
    W1dr = W1dr.reshape(128, B_PER * 2 * HID)

    # fc2 weights fp8, all k scaled 0.5 (Sign +-1 encoding), padded to 32
    # cols per k-tile. Layout [128, 4*32]: [p, k*32+o]
    w2t = W2.T.astype(np.float32)                     # [512, 20]
    W2pT = np.zeros((128, 4 * 32), dtype=FP8_NP)
    for k in range(4):
        W2pT[:, k * 32:k * 32 + OUT_DIM] = (
            0.5 * w2t[k * 128:(k + 1) * 128]).astype(FP8_NP)
    # effective (device) W2 after fp8 rounding, unscaled
    r2 = np.zeros(OUT_DIM, dtype=np.float64)
    for k in range(4):
        r2 += (W2pT[:, k * 32:k * 32 + OUT_DIM].astype(np.float64)
               .sum(axis=0)) / 0.5
    g = (1.0 - ALPHA ** (np.arange(T, dtype=np.float64) + 1)) / (1.0 - ALPHA)
    theta2 = (THETA - 0.5 * np.outer(r2, g)).astype(np.float32)   # [20, T]
    T2 = np.full((128, T), 3.0e4, dtype=np.float32)
    for b in range(B_PER):
        T2[32 * b:32 * b + OUT_DIM] = theta2
    T2 = T2.astype(BF16_NP)

    # Wl1T [128, KT*HID]: [p, k*HID+o] = Wl1[o, 128k+p]
    Wl1T = np.ascontiguousarray(
        Wl1.T.reshape(KT, 128, HID).transpose(1, 0, 2).reshape(128, KT * HID)
    ).astype(FP8_NP)

    # Wl2T [128, 4*32] bf16, 0.5-scaled (Sign +-1 l1 encoding), 32-col pad
    wl2t = Wl2.T.astype(np.float32)                   # [512, 20]
    Wl2T = np.zeros((128, 4 * 32), dtype=BF16_NP)
    for k in range(4):
        Wl2T[:, k * 32:k * 32 + OUT_DIM] = (
            0.5 * wl2t[k * 128:(k + 1) * 128]).astype(BF16_NP)
    r2l = np.zeros(OUT_DIM, dtype=np.float64)
    for k in range(4):
        r2l += (Wl2T[:, k * 32:k * 32 + OUT_DIM].astype(np.float64)
                .sum(axis=0)) / 0.5
    gcp = (1.0 - ALPHA ** (np.arange(CP, dtype=np.float64) + 1)) / (1.0 - ALPHA)
    theta2b = (THETA - 0.5 * np.outer(r2l, gcp)).astype(np.float32)  # [20,156]
    T2b = np.full((128, CP), 3.0e4, dtype=np.float32)
    for b in range(B_PER):
        T2b[32 * b:32 * b + OUT_DIM] = theta2b
    T2b = T2b.astype(BF16_NP)

    return {"W1dr": W1dr, "W2pT": W2pT, "Wl1T": Wl1T,
            "Wl2T": Wl2T, "T2": T2, "T2b": T2b}


def make_in_maps(spike_input, W1, W2, Wl1, Wl2, perm):
    si = np.asarray(spike_input, dtype=np.float32).reshape(B, C_IN, T)
    perm = np.asarray(perm).astype(np.int64)
    sip = si[:, perm, :]                              # perm-gather (layout only)
    shared = prep_shared_inputs(np.asarray(W1, np.float32),
                                np.asarray(W2, np.float32),
                                np.asarray(Wl1, np.float32),
                                np.asarray(Wl2, np.float32))
    in_maps = []
    for core in range(N_CORES):
        m = dict(shared)
        m.update(prep_core_inputs(si, sip, core))
        in_maps.append(m)
    return in_maps


_IN_SPECS = {
    "siA": ((128, B_PER * T), FP8),
    "siB": ((128, T), FP8),
    "sipT": ((128, KT * NB2), FP8),
    "W1dr": ((128, B_PER * 2 * HID), FP8),
    "W2pT": ((128, 4 * 32), FP8),
    "Wl1T": ((128, KT * HID), FP8),
    "Wl2T": ((128, 4 * 32), BF16),
    "T2": ((128, T), BF16),
    "T2b": ((128, CP), BF16),
}


def build_bass():
    nc = bacc.Bacc("TRN2", target_bir_lowering=False, debug=False)
    ins = {}
    for name, (shape, dt) in _IN_SPECS.items():
        h = nc.dram_tensor(name, list(shape), dt, kind="ExternalInput")
        ins[name] = h[:]
    out_h = nc.dram_tensor("out", [B_PER, 32, T + CP], BF16,
                           kind="ExternalOutput")
    outs = {"out": out_h[:]}
    with tile_mod.TileContext(nc) as tc:
        build_program(tc, outs, ins)
    nc.compile()
    return nc


_NC_CACHE = None


def run(inputs, trace=False, **kw):
    """Run on the 8 NeuronCores; returns (full_output, BassKernelResults)."""
    global _NC_CACHE
    if _NC_CACHE is None:
        _NC_CACHE = build_bass()
    nc = _NC_CACHE
    in_maps = make_in_maps(**inputs)
    res = run_bass_kernel_spmd(nc, in_maps, core_ids=list(range(N_CORES)),
                               trace=trace, **kw)
    parts = [res.results[c]["out"][:, :OUT_DIM, :] for c in range(N_CORES)]
    full = np.concatenate(parts, axis=0).reshape(B, OUT_DIM, 1, 1, T + CP)
    return np.ascontiguousarray(full.astype(np.float32)), res


def kernel(**inputs):
    out, _ = run(inputs)
    return out
